# revision 4
# baseline (speedup 1.0000x reference)
"""Multi-head causal attention (B=2, T=2048, C=1024, H=16) on 8 Trainium2
NeuronCores, tensor-parallel over heads (2 heads per core).

v2 layout/schedule (vs v1): engine-balanced flash-style pipeline.
  - chunks emitted batch-interleaved: (b0,q0),(b1,q0),(b0,q1),... so the
    Tile scheduler always has two independent attention streams.
  - phase 1 (QKV): QT/KT/VT [128, 4096] bf16 = w_c.T @ xT, 8 k-tiles;
    PSUM evacuation + bias on DVE (tensor_scalar_add), keeping ACT free.
  - phase 1.5: PE-transpose V (bf16) into vaug ([tok, dim] + ones column,
    66-col stride per head for 4B alignment).
  - phase 2 (scores): both heads' score matmuls (K=64) issued back-to-back
    into one [128, 1024] 2-bank PSUM tile -> they run concurrently via PE
    row tiling (row groups 0/64). Causal column-skipping: diagonal tile v
    only computes q-columns [128v:512).
  - phase 3 (softmax): ONE wide exp per k-tile over both heads
    (strided [128, 2, W] AP), PSUM->SBUF bf16. Causal masking applied
    AFTER exp as a 0/1 bf16 multiply on GPSIMD (the only engine with
    spare capacity), restricted to the [128, 2, 128] diagonal block.
  - phase 4 (AV): otps[h] [65, 512] += Vaug.T @ P (ones column => row 64
    accumulates softmax denominators), column-skipped, delayed AV_DELAY
    k-tiles behind the scores to hide exp latency.
  - normalize: sum rows extracted partition-aligned (ACT copy 64->64),
    broadcast via K=1 f32r matmul from partition 64, reciprocal + multiply
    on DVE. Head 1's normalized output is partition-shifted 0->64 by an
    SBUF->SBUF DMA so the out-projection contracts both heads at K=128.
  - phase 5 (out-proj): yT[m-tile, chunk] = wout.T @ ot, ONE matmul per
    m-tile (K=128); evacuation to bf16 on DVE; bf16 partials DMA'd out.
  - host: sum 8 bf16 partials in fp32, transpose, add b_out.
"""

import os
import sys

for _p in ("/opt/trn_rl_repo", "/root/.axon_site/_ro/trn_rl_repo"):
    if os.path.isdir(_p) and _p not in sys.path:
        sys.path.insert(0, _p)

import ml_dtypes
import numpy as np

import concourse.bacc as bacc
import concourse.bass as bass
import concourse.mybir as mybir
import concourse.tile as tile
from concourse.bass_utils import run_bass_kernel_spmd
from concourse.masks import make_identity

B, T, C, H, D = 2, 2048, 1024, 16, 64
NCORES = 8
BT = B * T                      # 4096 flattened tokens
TC = 512                        # token chunk (matmul free dim)
FP = mybir.dt.float32
FPR = mybir.dt.float32r
BF = mybir.dt.bfloat16
ACT = mybir.ActivationFunctionType
AV_DELAY = 2                    # k-tiles the AV matmul trails the scores

# chunk order: batch-interleaved
CHUNKS = [(b, qc) for qc in range(4) for b in range(B)]

LAST_RESULTS = None             # stashed BassKernelResults for test harness


def build_nc():
    nc = bacc.Bacc(None, target_bir_lowering=False, debug=False)

    xt = nc.declare_dram_parameter("xt", [C, BT], BF, isOutput=False)
    wc = nc.declare_dram_parameter("wc", [C, 384], BF, isOutput=False)
    wout = nc.declare_dram_parameter("wout", [128, C], BF, isOutput=False)
    bqkv = nc.declare_dram_parameter("bqkv", [128, 3], FP, isOutput=False)
    trimask = nc.declare_dram_parameter("trimask", [128, 256], BF, isOutput=False)
    ones = nc.declare_dram_parameter("ones", [128, 64], BF, isOutput=False)
    onesr = nc.declare_dram_parameter("onesr", [1, 64], FP, isOutput=False)
    yt = nc.declare_dram_parameter("yt", [C, BT], BF, isOutput=True)

    with tile.TileContext(nc) as tc:
        with (
            tc.tile_pool(name="const", bufs=1) as cpool,
            tc.tile_pool(name="big", bufs=1) as bigpool,
            tc.tile_pool(name="xs", bufs=24) as xpool,
            tc.tile_pool(name="sb", bufs=2) as sbpool,
            tc.tile_pool(name="w", bufs=2, space="PSUM") as wpool,
            tc.tile_pool(name="s", bufs=2, space="PSUM") as spool,
            tc.tile_pool(name="av", bufs=2, space="PSUM") as avpool,
        ):
            # ---- constants ----
            wc_sb = cpool.tile([128, 8 * 384], BF)      # [cin, k*384 + g*128 + col]
            nc.sync.dma_start(
                out=wc_sb[:].rearrange("b (a c) -> b a c", a=8),
                in_=wc.rearrange("(a b) c -> b a c", a=8),
            )
            wout_sb = cpool.tile([128, C], BF)
            nc.sync.dma_start(out=wout_sb[:], in_=wout[:, :])
            bq_sb = cpool.tile([128, 3], FP)
            nc.sync.dma_start(out=bq_sb[:], in_=bqkv[:, :])
            tri_sb = cpool.tile([128, 256], BF)
            nc.sync.dma_start(out=tri_sb[:], in_=trimask[:, :])
            onesr_sb = cpool.tile([65, 64], FPR)
            nc.sync.dma_start(out=onesr_sb[64:65, :], in_=onesr.bitcast(FPR)[:, :])
            identb = cpool.tile([128, 128], BF)
            make_identity(nc, identb)

            # ---- persistent intermediates ----
            QT = bigpool.tile([128, BT], BF)
            KT = bigpool.tile([128, BT], BF)
            VT = bigpool.tile([128, BT], BF)
            # V in [token, dim] layout, 132 cols per 128-token block:
            # [V_h0 (64) | ones | pad | V_h1 (64) | ones | pad]
            vaug = bigpool.tile([128, 32 * 132], BF)
            nc.sync.dma_start(
                out=vaug[:].rearrange("p (j a c) -> p j a c", a=2, c=66)[
                    :, :, :, 64:65],
                in_=ones.rearrange("p (j a c) -> p j a c", a=2, c=1)[:, 0:32],
            )

            qkvT = (QT, KT, VT)

            # input staging: prefetch 2 chunks ahead
            xts = {}

            def emit_x_dmas(ci):
                b, qc = CHUNKS[ci]
                t0 = b * 2048 + qc * TC
                for k in range(8):
                    xtile = xpool.tile([128, TC], BF, tag="xt")
                    nc.sync.dma_start(
                        out=xtile[:],
                        in_=xt[k * 128:(k + 1) * 128, t0:t0 + TC],
                    )
                    xts[(ci, k)] = xtile

            emit_x_dmas(0)
            emit_x_dmas(1)

            for ci, (b, qc) in enumerate(CHUNKS):
                if ci + 2 < len(CHUNKS):
                    emit_x_dmas(ci + 2)
                t0 = b * 2048 + qc * TC

                # ---- phase 1: QKV projection for this token chunk ----
                for g in range(3):
                    ps = wpool.tile([128, TC], FP, tag="w")
                    for k in range(8):
                        nc.tensor.matmul(
                            ps[:],
                            wc_sb[:, k * 384 + g * 128:k * 384 + (g + 1) * 128],
                            xts[(ci, k)][:],
                            start=(k == 0),
                            stop=(k == 7),
                        )
                    nc.vector.tensor_scalar_add(
                        qkvT[g][:, t0:t0 + TC], ps[:], bq_sb[:, g:g + 1]
                    )
                for k in range(8):
                    del xts[(ci, k)]

                # ---- phase 1.5: transpose this chunk's V into vaug ----
                for j in range(4):
                    jj = t0 // 128 + j
                    tp = wpool.tile([128, 128], BF, tag="w", name="tp")
                    nc.tensor.transpose(
                        tp[:], VT[:, jj * 128:(jj + 1) * 128], identb[:]
                    )
                    nc.vector.tensor_copy(
                        vaug[:].rearrange("p (j a c) -> p j a c", a=2, c=66)[
                            :, jj, :, 0:64],
                        tp[:].rearrange("p (a c) -> p a c", c=64),
                    )

                # ---- phase 2/3/4: causal attention for (b, qc) ----
                n_kt = 4 * (qc + 1)
                otps = [
                    avpool.tile([65, TC], FP, tag="av", name=f"otp{_h}")
                    for _h in range(2)
                ]
                pts = {}

                def emit_av(j, b=b, qc=qc, n_kt=n_kt, otps=otps, pts=pts):
                    kg = b * 16 + j
                    v = j - 4 * qc
                    off = 128 * v if v >= 0 else 0
                    pt = pts.pop(j)
                    for h in range(2):
                        nc.tensor.matmul(
                            otps[h][:, off:TC],
                            vaug[:, kg * 132 + h * 66:kg * 132 + h * 66 + 65],
                            pt[:, h * TC + off:(h + 1) * TC],
                            start=(j == 0), stop=(j == n_kt - 1),
                            skip_group_check=True,
                        )

                for kt in range(n_kt):
                    kg = b * 16 + kt
                    v = kt - 4 * qc
                    off = 128 * v if v >= 0 else 0
                    sp = spool.tile([128, 2 * TC], FP, tag="s")
                    for h in range(2):
                        nc.tensor.matmul(
                            sp[:, h * TC + off:(h + 1) * TC],
                            KT[h * 64:(h + 1) * 64, kg * 128:(kg + 1) * 128],
                            QT[h * 64:(h + 1) * 64, t0 + off:t0 + TC],
                            start=True, stop=True,
                        )
                    pt = xpool.tile([128, 2 * TC], BF, tag="pt", bufs=5)
                    if off:
                        nc.scalar.activation(
                            pt[:].rearrange("p (h q) -> p h q", h=2)[:, :, off:TC],
                            sp[:].rearrange("p (h q) -> p h q", h=2)[:, :, off:TC],
                            ACT.Exp, scale=0.125,
                        )
                    else:
                        nc.scalar.activation(pt[:], sp[:], ACT.Exp, scale=0.125)
                    if v >= 0:
                        ptv = pt[:].rearrange("p (h q) -> p h q", h=2)[
                            :, :, off:off + 128]
                        nc.gpsimd.tensor_tensor(
                            ptv,
                            ptv,
                            tri_sb[:].rearrange("p (h q) -> p h q", h=2),
                            mybir.AluOpType.mult,
                        )
                    pts[kt] = pt
                    if kt >= AV_DELAY:
                        emit_av(kt - AV_DELAY)
                for j in range(max(n_kt - AV_DELAY, 0), n_kt):
                    emit_av(j)

                # ---- normalize: 1/rowsum broadcast, per head ----
                rr = sbpool.tile([65, 2 * TC], FPR, tag="rr", name="rr")
                with nc.allow_low_precision(reason="softmax sums f32r"):
                    nc.scalar.copy(rr[64:65, 0:TC], otps[0][64:65, :])
                    nc.scalar.copy(rr[64:65, TC:2 * TC], otps[1][64:65, :])
                rcbs = []
                for h in range(2):
                    bch = wpool.tile([64, TC], FP, tag="w", name=f"bch{h}")
                    nc.tensor.matmul(
                        bch[:],
                        onesr_sb[64:65, :],
                        rr[64:65, h * TC:(h + 1) * TC],
                        start=True, stop=True,
                    )
                    rcb = sbpool.tile([64, TC], FP, tag="rcb", bufs=4,
                                      name=f"rcb{h}")
                    nc.vector.reciprocal_approx_fast(out=rcb[:], in_=bch[:])
                    rcbs.append(rcb)
                ot = sbpool.tile([128, TC], BF, tag="ot", name="ot")
                ot1 = sbpool.tile([64, TC], BF, tag="ot1", name="ot1")
                nc.vector.tensor_mul(ot[0:64, :], otps[0][0:64, :], rcbs[0][:])
                nc.vector.tensor_mul(ot1[:], otps[1][0:64, :], rcbs[1][:])
                nc.sync.dma_start(out=ot[64:128, :], in_=ot1[:])

                # ---- phase 5: output projection (contract both heads) ----
                for m in range(8):
                    yp = wpool.tile([128, TC], FP, tag="w", name="yp")
                    nc.tensor.matmul(
                        yp[:],
                        wout_sb[:, m * 128:(m + 1) * 128],
                        ot[:],
                        start=True, stop=True,
                    )
                    ys = sbpool.tile([128, TC], BF, tag="ys", bufs=6, name="ys")
                    nc.vector.tensor_copy(ys[:], yp[:])
                    nc.sync.dma_start(
                        out=yt[m * 128:(m + 1) * 128, t0:t0 + TC], in_=ys[:]
                    )
    nc.compile()
    return nc


def make_in_maps(x, w_qkv, b_qkv):
    x = np.ascontiguousarray(np.asarray(x, np.float32).reshape(BT, C))
    xT = np.ascontiguousarray(x.T).astype(ml_dtypes.bfloat16)
    w_qkv = np.asarray(w_qkv, np.float32)
    b_qkv = np.asarray(b_qkv, np.float32)

    # 0/1 triangular mask (k row <= q col within the diagonal 128-block),
    # two copies side by side for the paired-heads strided multiply
    kk = np.arange(128)[:, None]
    qq = np.arange(128)[None, :]
    tri = (kk <= qq).astype(np.float32)
    trimask = np.concatenate([tri, tri], axis=1).astype(ml_dtypes.bfloat16)

    in_maps = []
    for c in range(NCORES):
        sl = slice(c * 128, (c + 1) * 128)
        wcs = np.concatenate(
            [w_qkv[:, sl], w_qkv[:, 1024:][:, sl], w_qkv[:, 2048:][:, sl]], axis=1
        )
        bq = np.stack(
            [b_qkv[sl], b_qkv[1024:][sl], b_qkv[2048:][sl]], axis=1
        )
        in_maps.append({
            "xt": xT,
            "wc": np.ascontiguousarray(wcs).astype(ml_dtypes.bfloat16),
            "wout": None,  # filled by caller (needs w_out)
            "bqkv": np.ascontiguousarray(bq),
            "trimask": trimask,
            "ones": np.ones((128, 64), ml_dtypes.bfloat16),
            "onesr": np.ones((1, 64), np.float32),
        })
    return in_maps


_NC_CACHE = None


def kernel(x, w_qkv, b_qkv, w_out, b_out):
    global _NC_CACHE, LAST_RESULTS
    if _NC_CACHE is None:
        _NC_CACHE = build_nc()
    nc = _NC_CACHE

    w_out = np.asarray(w_out, np.float32)
    in_maps = make_in_maps(x, w_qkv, b_qkv)
    for c in range(NCORES):
        in_maps[c]["wout"] = np.ascontiguousarray(
            w_out[c * 128:(c + 1) * 128, :]).astype(ml_dtypes.bfloat16)

    res = run_bass_kernel_spmd(
        nc, in_maps, list(range(NCORES)),
        trace=bool(os.environ.get("BASS_TRACE")),
    )
    LAST_RESULTS = res

    acc = np.zeros((C, BT), np.float32)
    for out_map in res.results:
        acc += out_map["yt"].astype(np.float32)
    y = acc.T + np.asarray(b_out, np.float32)[None, :]
    return y.reshape(B, T, C)


# revision 7
# speedup vs baseline: 1.2567x; 1.2567x over previous
"""Multi-head causal attention (B=2, T=2048, C=1024, H=16) on 8 Trainium2
NeuronCores, tensor-parallel over heads (2 heads per core).

v2 layout/schedule (vs v1): engine-balanced flash-style pipeline.
  - chunks emitted batch-interleaved: (b0,q0),(b1,q0),(b0,q1),... so the
    Tile scheduler always has two independent attention streams.
  - phase 1 (QKV): QT/KT/VT [128, 4096] bf16 = w_c.T @ xT, 8 k-tiles;
    PSUM evacuation + bias on DVE (tensor_scalar_add), keeping ACT free.
  - phase 1.5: PE-transpose V (bf16) into vaug ([tok, dim] + ones column,
    66-col stride per head for 4B alignment).
  - phase 2 (scores): both heads' score matmuls (K=64) issued back-to-back
    into one [128, 1024] 2-bank PSUM tile -> they run concurrently via PE
    row tiling (row groups 0/64). Causal column-skipping: diagonal tile v
    only computes q-columns [128v:512).
  - phase 3 (softmax): ONE wide exp per k-tile over both heads
    (strided [128, 2, W] AP), PSUM->SBUF bf16. Causal masking applied
    AFTER exp as a 0/1 bf16 multiply on GPSIMD (the only engine with
    spare capacity), restricted to the [128, 2, 128] diagonal block.
  - phase 4 (AV): otps[h] [65, 512] += Vaug.T @ P (ones column => row 64
    accumulates softmax denominators), column-skipped, delayed AV_DELAY
    k-tiles behind the scores to hide exp latency.
  - normalize: sum rows extracted partition-aligned (ACT copy 64->64),
    broadcast via K=1 f32r matmul from partition 64, reciprocal + multiply
    on DVE. Head 1's normalized output is partition-shifted 0->64 by an
    SBUF->SBUF DMA so the out-projection contracts both heads at K=128.
  - phase 5 (out-proj): yT[m-tile, chunk] = wout.T @ ot, ONE matmul per
    m-tile (K=128); evacuation to bf16 on DVE; bf16 partials DMA'd out.
  - host: sum 8 bf16 partials in fp32, transpose, add b_out.
"""

import os
import sys

for _p in ("/opt/trn_rl_repo", "/root/.axon_site/_ro/trn_rl_repo"):
    if os.path.isdir(_p) and _p not in sys.path:
        sys.path.insert(0, _p)

import ml_dtypes
import numpy as np

import concourse.bacc as bacc
import concourse.bass as bass
import concourse.mybir as mybir
import concourse.tile as tile
from concourse.bass_utils import run_bass_kernel_spmd
from concourse.masks import make_identity

B, T, C, H, D = 2, 2048, 1024, 16, 64
NCORES = 8
BT = B * T                      # 4096 flattened tokens
TC = 512                        # token chunk (matmul free dim)
FP = mybir.dt.float32
FPR = mybir.dt.float32r
BF = mybir.dt.bfloat16
ACT = mybir.ActivationFunctionType
AV_DELAY = 2                    # k-tiles the AV matmul trails the scores

# chunk order: batch-interleaved
CHUNKS = [(b, qc) for qc in range(4) for b in range(B)]

LAST_RESULTS = None             # stashed BassKernelResults for test harness


def build_nc():
    nc = bacc.Bacc(None, target_bir_lowering=False, debug=False)

    xt = nc.declare_dram_parameter("xt", [C, BT], BF, isOutput=False)
    wc = nc.declare_dram_parameter("wc", [C, 384], BF, isOutput=False)
    wout = nc.declare_dram_parameter("wout", [128, C], BF, isOutput=False)
    bqkv = nc.declare_dram_parameter("bqkv", [128, 3], FP, isOutput=False)
    trimask = nc.declare_dram_parameter("trimask", [128, 256], BF, isOutput=False)
    ones = nc.declare_dram_parameter("ones", [128, 64], BF, isOutput=False)
    onesr = nc.declare_dram_parameter("onesr", [1, 64], FP, isOutput=False)
    yt = nc.declare_dram_parameter("yt", [C, BT], BF, isOutput=True)

    with tile.TileContext(nc) as tc:
        with (
            tc.tile_pool(name="const", bufs=1) as cpool,
            tc.tile_pool(name="big", bufs=1) as bigpool,
            tc.tile_pool(name="xs", bufs=24) as xpool,
            tc.tile_pool(name="sb", bufs=2) as sbpool,
            tc.tile_pool(name="w", bufs=2, space="PSUM") as wpool,
            tc.tile_pool(name="s", bufs=2, space="PSUM") as spool,
            tc.tile_pool(name="av", bufs=2, space="PSUM") as avpool,
        ):
            # ---- constants ----
            wc_sb = cpool.tile([128, 8 * 384], BF)      # [cin, k*384 + g*128 + col]
            nc.sync.dma_start(
                out=wc_sb[:].rearrange("b (a c) -> b a c", a=8),
                in_=wc.rearrange("(a b) c -> b a c", a=8),
            )
            wout_sb = cpool.tile([128, C], BF)
            nc.sync.dma_start(out=wout_sb[:], in_=wout[:, :])
            bq_sb = cpool.tile([128, 3], FP)
            nc.sync.dma_start(out=bq_sb[:], in_=bqkv[:, :])
            tri_sb = cpool.tile([128, 256], BF)
            nc.sync.dma_start(out=tri_sb[:], in_=trimask[:, :])
            onesr_sb = cpool.tile([65, 64], FPR)
            nc.sync.dma_start(out=onesr_sb[64:65, :], in_=onesr.bitcast(FPR)[:, :])
            identb = cpool.tile([128, 128], BF)
            make_identity(nc, identb)

            # ---- persistent intermediates ----
            QT = bigpool.tile([128, BT], BF)
            KT = bigpool.tile([128, BT], BF)
            VT = bigpool.tile([128, BT], BF)
            # V in [token, dim] layout, 132 cols per 128-token block:
            # [V_h0 (64) | ones | pad | V_h1 (64) | ones | pad]
            vaug = bigpool.tile([128, 32 * 132], BF)
            nc.sync.dma_start(
                out=vaug[:].rearrange("p (j a c) -> p j a c", a=2, c=66)[
                    :, :, :, 64:65],
                in_=ones.rearrange("p (j a c) -> p j a c", a=2, c=1)[:, 0:32],
            )

            qkvT = (QT, KT, VT)

            # input staging: prefetch 2 chunks ahead
            xts = {}

            def emit_x_dmas(ci):
                b, qc = CHUNKS[ci]
                t0 = b * 2048 + qc * TC
                for k in range(8):
                    xtile = xpool.tile([128, TC], BF, tag="xt")
                    nc.sync.dma_start(
                        out=xtile[:],
                        in_=xt[k * 128:(k + 1) * 128, t0:t0 + TC],
                    )
                    xts[(ci, k)] = xtile

            emit_x_dmas(0)
            emit_x_dmas(1)

            for ci, (b, qc) in enumerate(CHUNKS):
                if ci + 2 < len(CHUNKS):
                    emit_x_dmas(ci + 2)
                t0 = b * 2048 + qc * TC

                # ---- phase 1: QKV projection for this token chunk ----
                for g in range(3):
                    ps = wpool.tile([128, TC], FP, tag="w")
                    for k in range(8):
                        nc.tensor.matmul(
                            ps[:],
                            wc_sb[:, k * 384 + g * 128:k * 384 + (g + 1) * 128],
                            xts[(ci, k)][:],
                            start=(k == 0),
                            stop=(k == 7),
                        )
                    nc.vector.tensor_scalar_add(
                        qkvT[g][:, t0:t0 + TC], ps[:], bq_sb[:, g:g + 1]
                    )
                for k in range(8):
                    del xts[(ci, k)]

                # ---- phase 1.5: transpose this chunk's V into vaug ----
                for j in range(4):
                    jj = t0 // 128 + j
                    tp = spool.tile([128, 128], BF, tag="s", name="tp")
                    nc.tensor.transpose(
                        tp[:], VT[:, jj * 128:(jj + 1) * 128], identb[:]
                    )
                    nc.vector.tensor_copy(
                        vaug[:].rearrange("p (j a c) -> p j a c", a=2, c=66)[
                            :, jj, :, 0:64],
                        tp[:].rearrange("p (a c) -> p a c", c=64),
                    )

                # ---- phase 2/3/4: causal attention for (b, qc) ----
                n_kt = 4 * (qc + 1)
                otps = [
                    avpool.tile([65, TC], FP, tag="av", name=f"otp{_h}")
                    for _h in range(2)
                ]
                pts = {}

                def emit_av(j, b=b, qc=qc, n_kt=n_kt, otps=otps, pts=pts):
                    kg = b * 16 + j
                    v = j - 4 * qc
                    off = 128 * v if v >= 0 else 0
                    pt = pts.pop(j)
                    for h in range(2):
                        nc.tensor.matmul(
                            otps[h][:, off:TC],
                            vaug[:, kg * 132 + h * 66:kg * 132 + h * 66 + 65],
                            pt[:, h * TC + off:(h + 1) * TC],
                            start=(j == 0), stop=(j == n_kt - 1),
                            skip_group_check=True,
                        )

                for kt in range(n_kt):
                    kg = b * 16 + kt
                    v = kt - 4 * qc
                    off = 128 * v if v >= 0 else 0
                    sp = spool.tile([128, 2 * TC], FP, tag="s")
                    for h in range(2):
                        nc.tensor.matmul(
                            sp[:, h * TC + off:(h + 1) * TC],
                            KT[h * 64:(h + 1) * 64, kg * 128:(kg + 1) * 128],
                            QT[h * 64:(h + 1) * 64, t0 + off:t0 + TC],
                            start=True, stop=True,
                        )
                    pt = xpool.tile([128, 2 * TC], BF, tag="pt", bufs=5)
                    if off:
                        nc.scalar.activation(
                            pt[:].rearrange("p (h q) -> p h q", h=2)[:, :, off:TC],
                            sp[:].rearrange("p (h q) -> p h q", h=2)[:, :, off:TC],
                            ACT.Exp, scale=0.125,
                        )
                    else:
                        nc.scalar.activation(pt[:], sp[:], ACT.Exp, scale=0.125)
                    if v >= 0:
                        ptv = pt[:].rearrange("p (h q) -> p h q", h=2)[
                            :, :, off:off + 128]
                        nc.gpsimd.tensor_tensor(
                            ptv,
                            ptv,
                            tri_sb[:].rearrange("p (h q) -> p h q", h=2),
                            mybir.AluOpType.mult,
                        )
                    pts[kt] = pt
                    if kt >= AV_DELAY:
                        emit_av(kt - AV_DELAY)
                for j in range(max(n_kt - AV_DELAY, 0), n_kt):
                    emit_av(j)

                # ---- normalize: 1/rowsum broadcast, per head ----
                rr = sbpool.tile([65, 2 * TC], FPR, tag="rr", name="rr")
                with nc.allow_low_precision(reason="softmax sums f32r"):
                    nc.scalar.copy(rr[64:65, 0:TC], otps[0][64:65, :])
                    nc.scalar.copy(rr[64:65, TC:2 * TC], otps[1][64:65, :])
                rcbs = []
                for h in range(2):
                    bch = spool.tile([64, TC], FP, tag="s", name=f"bch{h}")
                    nc.tensor.matmul(
                        bch[:],
                        onesr_sb[64:65, :],
                        rr[64:65, h * TC:(h + 1) * TC],
                        start=True, stop=True,
                    )
                    rcb = sbpool.tile([64, TC], FP, tag="rcb", bufs=4,
                                      name=f"rcb{h}")
                    nc.vector.reciprocal_approx_fast(out=rcb[:], in_=bch[:])
                    rcbs.append(rcb)
                ot = sbpool.tile([128, TC], BF, tag="ot", name="ot")
                ot1 = sbpool.tile([64, TC], BF, tag="ot1", name="ot1")
                nc.vector.tensor_mul(ot[0:64, :], otps[0][0:64, :], rcbs[0][:])
                nc.vector.tensor_mul(ot1[:], otps[1][0:64, :], rcbs[1][:])
                nc.sync.dma_start(out=ot[64:128, :], in_=ot1[:])

                # ---- phase 5: output projection (contract both heads) ----
                for m in range(8):
                    yp = avpool.tile([128, TC], FP, tag="av", name="yp")
                    nc.tensor.matmul(
                        yp[:],
                        wout_sb[:, m * 128:(m + 1) * 128],
                        ot[:],
                        start=True, stop=True,
                    )
                    ys = sbpool.tile([128, TC], BF, tag="ys", bufs=6, name="ys")
                    nc.vector.tensor_copy(ys[:], yp[:])
                    nc.sync.dma_start(
                        out=yt[m * 128:(m + 1) * 128, t0:t0 + TC], in_=ys[:]
                    )
    nc.compile()
    return nc


def make_in_maps(x, w_qkv, b_qkv):
    x = np.ascontiguousarray(np.asarray(x, np.float32).reshape(BT, C))
    xT = np.ascontiguousarray(x.T).astype(ml_dtypes.bfloat16)
    w_qkv = np.asarray(w_qkv, np.float32)
    b_qkv = np.asarray(b_qkv, np.float32)

    # 0/1 triangular mask (k row <= q col within the diagonal 128-block),
    # two copies side by side for the paired-heads strided multiply
    kk = np.arange(128)[:, None]
    qq = np.arange(128)[None, :]
    tri = (kk <= qq).astype(np.float32)
    trimask = np.concatenate([tri, tri], axis=1).astype(ml_dtypes.bfloat16)

    in_maps = []
    for c in range(NCORES):
        sl = slice(c * 128, (c + 1) * 128)
        wcs = np.concatenate(
            [w_qkv[:, sl], w_qkv[:, 1024:][:, sl], w_qkv[:, 2048:][:, sl]], axis=1
        )
        bq = np.stack(
            [b_qkv[sl], b_qkv[1024:][sl], b_qkv[2048:][sl]], axis=1
        )
        in_maps.append({
            "xt": xT,
            "wc": np.ascontiguousarray(wcs).astype(ml_dtypes.bfloat16),
            "wout": None,  # filled by caller (needs w_out)
            "bqkv": np.ascontiguousarray(bq),
            "trimask": trimask,
            "ones": np.ones((128, 64), ml_dtypes.bfloat16),
            "onesr": np.ones((1, 64), np.float32),
        })
    return in_maps


_NC_CACHE = None


def kernel(x, w_qkv, b_qkv, w_out, b_out):
    global _NC_CACHE, LAST_RESULTS
    if _NC_CACHE is None:
        _NC_CACHE = build_nc()
    nc = _NC_CACHE

    w_out = np.asarray(w_out, np.float32)
    in_maps = make_in_maps(x, w_qkv, b_qkv)
    for c in range(NCORES):
        in_maps[c]["wout"] = np.ascontiguousarray(
            w_out[c * 128:(c + 1) * 128, :]).astype(ml_dtypes.bfloat16)

    res = run_bass_kernel_spmd(
        nc, in_maps, list(range(NCORES)),
        trace=bool(os.environ.get("BASS_TRACE")),
    )
    LAST_RESULTS = res

    acc = np.zeros((C, BT), np.float32)
    for out_map in res.results:
        acc += out_map["yt"].astype(np.float32)
    y = acc.T + np.asarray(b_out, np.float32)[None, :]
    return y.reshape(B, T, C)


# revision 8
# speedup vs baseline: 1.2941x; 1.0297x over previous
"""Multi-head causal attention (B=2, T=2048, C=1024, H=16) on 8 Trainium2
NeuronCores, tensor-parallel over heads (2 heads per core).

v2 layout/schedule (vs v1): engine-balanced flash-style pipeline.
  - chunks emitted batch-interleaved: (b0,q0),(b1,q0),(b0,q1),... so the
    Tile scheduler always has two independent attention streams.
  - phase 1 (QKV): QT/KT/VT [128, 4096] bf16 = w_c.T @ xT, 8 k-tiles;
    PSUM evacuation + bias on DVE (tensor_scalar_add), keeping ACT free.
  - phase 1.5: PE-transpose V (bf16) into vaug ([tok, dim] + ones column,
    66-col stride per head for 4B alignment).
  - phase 2 (scores): both heads' score matmuls (K=64) issued back-to-back
    into one [128, 1024] 2-bank PSUM tile -> they run concurrently via PE
    row tiling (row groups 0/64). Causal column-skipping: diagonal tile v
    only computes q-columns [128v:512).
  - phase 3 (softmax): ONE wide exp per k-tile over both heads
    (strided [128, 2, W] AP), PSUM->SBUF bf16. Causal masking applied
    AFTER exp as a 0/1 bf16 multiply on GPSIMD (the only engine with
    spare capacity), restricted to the [128, 2, 128] diagonal block.
  - phase 4 (AV): otps[h] [65, 512] += Vaug.T @ P (ones column => row 64
    accumulates softmax denominators), column-skipped, delayed AV_DELAY
    k-tiles behind the scores to hide exp latency.
  - normalize: sum rows extracted partition-aligned (ACT copy 64->64),
    broadcast via K=1 f32r matmul from partition 64, reciprocal + multiply
    on DVE. Head 1's normalized output is partition-shifted 0->64 by an
    SBUF->SBUF DMA so the out-projection contracts both heads at K=128.
  - phase 5 (out-proj): yT[m-tile, chunk] = wout.T @ ot, ONE matmul per
    m-tile (K=128); evacuation to bf16 on DVE; bf16 partials DMA'd out.
  - host: sum 8 bf16 partials in fp32, transpose, add b_out.
"""

import os
import sys

for _p in ("/opt/trn_rl_repo", "/root/.axon_site/_ro/trn_rl_repo"):
    if os.path.isdir(_p) and _p not in sys.path:
        sys.path.insert(0, _p)

import ml_dtypes
import numpy as np

import concourse.bacc as bacc
import concourse.bass as bass
import concourse.mybir as mybir
import concourse.tile as tile
from concourse.bass_utils import run_bass_kernel_spmd
from concourse.masks import make_identity

B, T, C, H, D = 2, 2048, 1024, 16, 64
NCORES = 8
BT = B * T                      # 4096 flattened tokens
TC = 512                        # token chunk (matmul free dim)
FP = mybir.dt.float32
FPR = mybir.dt.float32r
BF = mybir.dt.bfloat16
ACT = mybir.ActivationFunctionType
AV_DELAY = 2                    # k-tiles the AV matmul trails the scores

# chunk order: batch-interleaved
CHUNKS = [(b, qc) for qc in range(4) for b in range(B)]

LAST_RESULTS = None             # stashed BassKernelResults for test harness


def build_nc():
    nc = bacc.Bacc(None, target_bir_lowering=False, debug=False)

    xt = nc.declare_dram_parameter("xt", [C, BT], BF, isOutput=False)
    wc = nc.declare_dram_parameter("wc", [C, 384], BF, isOutput=False)
    wout = nc.declare_dram_parameter("wout", [128, C], BF, isOutput=False)
    bqkv = nc.declare_dram_parameter("bqkv", [128, 3], FP, isOutput=False)
    trimask = nc.declare_dram_parameter("trimask", [128, 256], BF, isOutput=False)
    ones = nc.declare_dram_parameter("ones", [128, 64], BF, isOutput=False)
    onesr = nc.declare_dram_parameter("onesr", [1, 64], FP, isOutput=False)
    yt = nc.declare_dram_parameter("yt", [C, BT], BF, isOutput=True)

    with tile.TileContext(nc) as tc:
        with (
            tc.tile_pool(name="const", bufs=1) as cpool,
            tc.tile_pool(name="big", bufs=1) as bigpool,
            tc.tile_pool(name="xs", bufs=24) as xpool,
            tc.tile_pool(name="sb", bufs=2) as sbpool,
            tc.tile_pool(name="w", bufs=2, space="PSUM") as wpool,
            tc.tile_pool(name="s", bufs=2, space="PSUM") as spool,
            tc.tile_pool(name="av", bufs=2, space="PSUM") as avpool,
        ):
            # ---- constants ----
            wc_sb = cpool.tile([128, 8 * 384], BF)      # [cin, k*384 + g*128 + col]
            nc.sync.dma_start(
                out=wc_sb[:].rearrange("b (a c) -> b a c", a=8),
                in_=wc.rearrange("(a b) c -> b a c", a=8),
            )
            wout_sb = cpool.tile([128, C], BF)
            nc.sync.dma_start(out=wout_sb[:], in_=wout[:, :])
            bq_sb = cpool.tile([128, 3], FP)
            nc.sync.dma_start(out=bq_sb[:], in_=bqkv[:, :])
            tri_sb = cpool.tile([128, 256], BF)
            nc.sync.dma_start(out=tri_sb[:], in_=trimask[:, :])
            onesr_sb = cpool.tile([65, 64], FPR)
            nc.sync.dma_start(out=onesr_sb[64:65, :], in_=onesr.bitcast(FPR)[:, :])
            identb = cpool.tile([128, 128], BF)
            make_identity(nc, identb)

            # ---- persistent intermediates ----
            QT = bigpool.tile([128, BT], BF)
            KT = bigpool.tile([128, BT], BF)
            VT = bigpool.tile([128, BT], BF)
            # V in [token, dim] layout, 132 cols per 128-token block:
            # [V_h0 (64) | ones | pad | V_h1 (64) | ones | pad]
            vaug = bigpool.tile([128, 32 * 132], BF)
            nc.sync.dma_start(
                out=vaug[:].rearrange("p (j a c) -> p j a c", a=2, c=66)[
                    :, :, :, 64:65],
                in_=ones.rearrange("p (j a c) -> p j a c", a=2, c=1)[:, 0:32],
            )

            qkvT = (QT, KT, VT)

            # input staging: prefetch 2 chunks ahead
            xts = {}

            def emit_x_dmas(ci):
                b, qc = CHUNKS[ci]
                t0 = b * 2048 + qc * TC
                for k in range(8):
                    xtile = xpool.tile([128, TC], BF, tag="xt")
                    nc.gpsimd.dma_start(
                        out=xtile[:],
                        in_=xt[k * 128:(k + 1) * 128, t0:t0 + TC],
                    )
                    xts[(ci, k)] = xtile

            emit_x_dmas(0)
            emit_x_dmas(1)

            for ci, (b, qc) in enumerate(CHUNKS):
                if ci + 2 < len(CHUNKS):
                    emit_x_dmas(ci + 2)
                t0 = b * 2048 + qc * TC

                # ---- phase 1: QKV projection for this token chunk ----
                for g in range(3):
                    ps = wpool.tile([128, TC], FP, tag="w")
                    for k in range(8):
                        nc.tensor.matmul(
                            ps[:],
                            wc_sb[:, k * 384 + g * 128:k * 384 + (g + 1) * 128],
                            xts[(ci, k)][:],
                            start=(k == 0),
                            stop=(k == 7),
                        )
                    nc.vector.tensor_scalar_add(
                        qkvT[g][:, t0:t0 + TC], ps[:], bq_sb[:, g:g + 1]
                    )
                for k in range(8):
                    del xts[(ci, k)]

                # ---- phase 1.5: transpose this chunk's V into vaug ----
                for j in range(4):
                    jj = t0 // 128 + j
                    tp = spool.tile([128, 128], BF, tag="s", name="tp")
                    nc.tensor.transpose(
                        tp[:], VT[:, jj * 128:(jj + 1) * 128], identb[:]
                    )
                    nc.vector.tensor_copy(
                        vaug[:].rearrange("p (j a c) -> p j a c", a=2, c=66)[
                            :, jj, :, 0:64],
                        tp[:].rearrange("p (a c) -> p a c", c=64),
                    )

                # ---- phase 2/3/4: causal attention for (b, qc) ----
                n_kt = 4 * (qc + 1)
                otps = [
                    avpool.tile([65, TC], FP, tag="av", name=f"otp{_h}")
                    for _h in range(2)
                ]
                pts = {}

                def emit_av(j, b=b, qc=qc, n_kt=n_kt, otps=otps, pts=pts):
                    kg = b * 16 + j
                    v = j - 4 * qc
                    off = 128 * v if v >= 0 else 0
                    pt = pts.pop(j)
                    for h in range(2):
                        nc.tensor.matmul(
                            otps[h][:, off:TC],
                            vaug[:, kg * 132 + h * 66:kg * 132 + h * 66 + 65],
                            pt[:, h * TC + off:(h + 1) * TC],
                            start=(j == 0), stop=(j == n_kt - 1),
                            skip_group_check=True,
                        )

                for kt in range(n_kt):
                    kg = b * 16 + kt
                    v = kt - 4 * qc
                    off = 128 * v if v >= 0 else 0
                    sp = spool.tile([128, 2 * TC], FP, tag="s")
                    for h in range(2):
                        nc.tensor.matmul(
                            sp[:, h * TC + off:(h + 1) * TC],
                            KT[h * 64:(h + 1) * 64, kg * 128:(kg + 1) * 128],
                            QT[h * 64:(h + 1) * 64, t0 + off:t0 + TC],
                            start=True, stop=True,
                        )
                    pt = xpool.tile([128, 2 * TC], BF, tag="pt", bufs=5)
                    if off:
                        nc.scalar.activation(
                            pt[:].rearrange("p (h q) -> p h q", h=2)[:, :, off:TC],
                            sp[:].rearrange("p (h q) -> p h q", h=2)[:, :, off:TC],
                            ACT.Exp, scale=0.125,
                        )
                    else:
                        nc.scalar.activation(pt[:], sp[:], ACT.Exp, scale=0.125)
                    if v >= 0:
                        ptv = pt[:].rearrange("p (h q) -> p h q", h=2)[
                            :, :, off:off + 128]
                        nc.gpsimd.tensor_tensor(
                            ptv,
                            ptv,
                            tri_sb[:].rearrange("p (h q) -> p h q", h=2),
                            mybir.AluOpType.mult,
                        )
                    pts[kt] = pt
                    if kt >= AV_DELAY:
                        emit_av(kt - AV_DELAY)
                for j in range(max(n_kt - AV_DELAY, 0), n_kt):
                    emit_av(j)

                # ---- normalize: 1/rowsum broadcast, per head ----
                rr = sbpool.tile([65, 2 * TC], FPR, tag="rr", name="rr")
                with nc.allow_low_precision(reason="softmax sums f32r"):
                    nc.scalar.copy(rr[64:65, 0:TC], otps[0][64:65, :])
                    nc.scalar.copy(rr[64:65, TC:2 * TC], otps[1][64:65, :])
                rcbs = []
                for h in range(2):
                    bch = spool.tile([64, TC], FP, tag="s", name=f"bch{h}")
                    nc.tensor.matmul(
                        bch[:],
                        onesr_sb[64:65, :],
                        rr[64:65, h * TC:(h + 1) * TC],
                        start=True, stop=True,
                    )
                    rcb = sbpool.tile([64, TC], FP, tag="rcb", bufs=4,
                                      name=f"rcb{h}")
                    nc.vector.reciprocal_approx_fast(out=rcb[:], in_=bch[:])
                    rcbs.append(rcb)
                ot = sbpool.tile([128, TC], BF, tag="ot", name="ot")
                ot1 = sbpool.tile([64, TC], BF, tag="ot1", name="ot1")
                nc.vector.tensor_mul(ot[0:64, :], otps[0][0:64, :], rcbs[0][:])
                nc.vector.tensor_mul(ot1[:], otps[1][0:64, :], rcbs[1][:])
                nc.sync.dma_start(out=ot[64:128, :], in_=ot1[:])

                # ---- phase 5: output projection (contract both heads) ----
                for m in range(8):
                    yp = avpool.tile([128, TC], FP, tag="av", name="yp")
                    nc.tensor.matmul(
                        yp[:],
                        wout_sb[:, m * 128:(m + 1) * 128],
                        ot[:],
                        start=True, stop=True,
                    )
                    ys = sbpool.tile([128, TC], BF, tag="ys", bufs=6, name="ys")
                    nc.vector.tensor_copy(ys[:], yp[:])
                    nc.sync.dma_start(
                        out=yt[m * 128:(m + 1) * 128, t0:t0 + TC], in_=ys[:]
                    )
    nc.compile()
    return nc


def make_in_maps(x, w_qkv, b_qkv):
    x = np.ascontiguousarray(np.asarray(x, np.float32).reshape(BT, C))
    xT = np.ascontiguousarray(x.T).astype(ml_dtypes.bfloat16)
    w_qkv = np.asarray(w_qkv, np.float32)
    b_qkv = np.asarray(b_qkv, np.float32)

    # 0/1 triangular mask (k row <= q col within the diagonal 128-block),
    # two copies side by side for the paired-heads strided multiply
    kk = np.arange(128)[:, None]
    qq = np.arange(128)[None, :]
    tri = (kk <= qq).astype(np.float32)
    trimask = np.concatenate([tri, tri], axis=1).astype(ml_dtypes.bfloat16)

    in_maps = []
    for c in range(NCORES):
        sl = slice(c * 128, (c + 1) * 128)
        wcs = np.concatenate(
            [w_qkv[:, sl], w_qkv[:, 1024:][:, sl], w_qkv[:, 2048:][:, sl]], axis=1
        )
        bq = np.stack(
            [b_qkv[sl], b_qkv[1024:][sl], b_qkv[2048:][sl]], axis=1
        )
        in_maps.append({
            "xt": xT,
            "wc": np.ascontiguousarray(wcs).astype(ml_dtypes.bfloat16),
            "wout": None,  # filled by caller (needs w_out)
            "bqkv": np.ascontiguousarray(bq),
            "trimask": trimask,
            "ones": np.ones((128, 64), ml_dtypes.bfloat16),
            "onesr": np.ones((1, 64), np.float32),
        })
    return in_maps


_NC_CACHE = None


def kernel(x, w_qkv, b_qkv, w_out, b_out):
    global _NC_CACHE, LAST_RESULTS
    if _NC_CACHE is None:
        _NC_CACHE = build_nc()
    nc = _NC_CACHE

    w_out = np.asarray(w_out, np.float32)
    in_maps = make_in_maps(x, w_qkv, b_qkv)
    for c in range(NCORES):
        in_maps[c]["wout"] = np.ascontiguousarray(
            w_out[c * 128:(c + 1) * 128, :]).astype(ml_dtypes.bfloat16)

    res = run_bass_kernel_spmd(
        nc, in_maps, list(range(NCORES)),
        trace=bool(os.environ.get("BASS_TRACE")),
    )
    LAST_RESULTS = res

    acc = np.zeros((C, BT), np.float32)
    for out_map in res.results:
        acc += out_map["yt"].astype(np.float32)
    y = acc.T + np.asarray(b_out, np.float32)[None, :]
    return y.reshape(B, T, C)
